# revision 38
# baseline (speedup 1.0000x reference)
"""Trainium2 Bass kernel for BatchLabelPropagation.

Per episode b (of 16), e=128 samples, c=512 channels:
  sq = ||x_i - x_j||^2 / sqrt(c)   (pairwise, diag exactly 0)
  standardize sq by GLOBAL masked std (ddof=1); W = exp(-sq~), diag zeroed
  S = W * colscale_j; P = inv(I - 0.2 S); P rows L1-normalized
  out = log(P @ onehot + 1e-6)

Scale-free device formulation: the device only ever computes
  sqhalf = G - (r_i + r_j)/2          (= -sq * sqrt(c) / 2)
and W = exp(s * sqhalf) with the single host-combined scalar
  s = 1/sqrt(var(sqhalf)) -- all 1/sqrt(c) factors cancel.

8 NeuronCores, 2 episodes/core, data parallel.  Two launches with a
tiny host-side stats combine between them (the global variance needs
all 16 episodes; a device collective has a ~20us floor on real HW):

  L1: xt (c-major, bf16) -> Gram on PE (4 bf16 chunk matmuls) plus a 5th
     K=4 bf16 "augmentation" matmul accumulating -(r_i+r_j)/2 into the
     same PSUM tile.  r = rowsum(x^2) is computed on the HOST from the
     bf16-quantized x (input marshalling) and shipped as a bf16 hi+lo
     pair so the augmentation is f32-exact.  The two PSUM tiles are
     copied into one SBUF tile (DVE) and shipped in a single DMA.
     No ACT ops at all -> no activation-table loads.

  host: A = sum(sqhalf), Qu = sum(sqhalf^2) in f64 over the shipped
     tiles; var = (Qu - A^2/cnt)/(cnt-1); s = 1/sqrt(var).

  L2: one input DMA carries [s | sqhalf | B] per core; the host zeroes
     the sqhalf diagonal exactly during the roundtrip (the PE leaves a
     ~1e-3 residue that would poison the 1e-4 damping), so W_ii == 1
     bit-exact and no mask matrix is ever needed.  A manual
     LoadActFuncSet(6) makes exp/ln/copy resident in ONE table load
     (hidden behind the input DMA).  W = exp(s*sqhalf) in bf16 (the bf16
     Gram noise dominates the error budget): ep1's Exp carries the
     rowsum via fused accum_out, ep0's rowsum runs on the idle DVE so
     the serial ACT queue finishes ep1's accum-read 187ns earlier.
     Neumann series (2 terms, fro ~5e-4): u0 = t*B,
     u_{k+1} = t*((W-I)@u_k) + u0 where (W-I)@u is W@u + (-I)@u in one
     PSUM group and the fused DVE scalar_tensor_tensor doubles as the
     PSUM->SBUF move; the last group also accumulates I@B so
     v = B + (W-I)@u_last lands complete in PSUM.
     out = Ln(v[:,0:5] * (1/v[:,5]) + 1e-6) in one ACT op per episode.
"""
import numpy as np
import ml_dtypes

import concourse.bass as bass
import concourse.bacc as bacc
import concourse.tile as tile
from concourse import mybir
from concourse import bass_utils

NCORES = 8
B_FULL = 16
EP = B_FULL // NCORES  # episodes per core
E = 128
C = 512
KCHUNKS = C // 128
NCLASSES = 5
NB = NCLASSES + 1
SQW = EP * E  # sqhalf columns in the packed L1 output

ALPHA = 0.2
EPS_OUT = 1e-6
EPS_DIAG = 1e-4
NEUMANN_ITERS = 2
CNT = float(B_FULL * E * (E - 1))

F32 = mybir.dt.float32
BF16 = mybir.dt.bfloat16
AF = mybir.ActivationFunctionType
ALU = mybir.AluOpType
AX = mybir.AxisListType

_CACHE = {}


def _new_bacc(ncores):
    return bacc.Bacc(
        "TRN2",
        target_bir_lowering=False,
        debug=False,
        enable_asserts=True,
        num_devices=ncores,
    )


def _load_act_set(nc, set_id):
    """Manually pin an ACT function table at the top of the program so the
    compiler's greedy insertion pass adds no mid-kernel reloads."""
    nc.scalar.add_instruction(
        mybir.InstLoadActFuncSet(
            name=nc.get_next_instruction_name(),
            act_func_set_id=set_id,
            ins=[],
            outs=[],
        )
    )


def _build_l1(ncores=NCORES):
    nc = _new_bacc(ncores)
    # xt[p, ep, k, e] = bf16(x[ep, e, 128*k + p]) -- 1KB contiguous per
    # (partition, episode) so each episode is a 128-descriptor DMA.
    xt_d = nc.dram_tensor("xt", [E, EP, KCHUNKS, E], BF16, kind="ExternalInput").ap()
    # aug[k, ep, side, e]: K=4 augmentation operands (see _prepare_l1_in_maps)
    aug_d = nc.dram_tensor("aug", [4, EP, 2, E], BF16, kind="ExternalInput").ap()
    out_d = nc.dram_tensor("sqh", [E, SQW], F32, kind="ExternalOutput").ap()

    with tile.TileContext(nc) as tc:
        with (
            tc.tile_pool(name="sb", bufs=1) as sb,
            tc.tile_pool(name="ps", bufs=1, space="PSUM") as ps,
        ):
            # inputs: both episodes on the SP HWDGE queue (descriptor gen is
            # a single shared resource, and SP has the fastest DGE delay);
            # aug rides the Pool SWDGE queue in parallel
            xt0 = sb.tile([E, KCHUNKS, E], BF16, tag="xt0")
            nc.sync.dma_start(out=xt0, in_=xt_d[:, 0])
            xt1 = sb.tile([E, KCHUNKS, E], BF16, tag="xt1")
            nc.sync.dma_start(out=xt1, in_=xt_d[:, 1])
            aug = sb.tile([4, EP, 2, E], BF16, tag="aug")
            nc.gpsimd.dma_start(out=aug, in_=aug_d)
            xts = (xt0, xt1)

            # separate per-episode output tiles so the two halves ship on
            # independent queues as soon as each copy lands
            out0 = sb.tile([E, E], F32, tag="o0")
            out1 = sb.tile([E, E], F32, tag="o1")
            outs = [out0, out1]

            for ep in range(EP):
                g = ps.tile([E, E], F32, tag=f"g{ep}")
                for k in range(KCHUNKS):
                    ck = xts[ep][:, k, :]
                    nc.tensor.matmul(g, ck, ck, start=(k == 0), stop=False)
                # sqhalf = G - r_i/2 - r_j/2 via one K=4 accumulation:
                # lhsT = [1;1;rhi;rlo], rhs = [rhi;rlo;1;1]
                nc.tensor.matmul(
                    g, aug[:, ep, 0, :], aug[:, ep, 1, :], start=False, stop=True
                )
                nc.vector.tensor_copy(outs[ep], g)

            nc.sync.dma_start(out=out_d[:, 0:E], in_=outs[0])
            nc.sync.dma_start(out=out_d[:, E : 2 * E], in_=outs[1])

    nc.compile()
    return nc


def _build_l2(ncores=NCORES):
    nc = _new_bacc(ncores)
    # one input DMA: [s | sqhalf(2 eps) | B(2 eps x 6)]
    INW = 1 + SQW + EP * NB
    in_d = nc.dram_tensor("sqs", [E, INW], F32, kind="ExternalInput").ap()
    out_d = nc.dram_tensor("out", [E, EP, NCLASSES], F32, kind="ExternalOutput").ap()

    # [-eye | +eye] bf16: the Neumann matmul subtracts u and re-adds B
    eyes_np = np.concatenate(
        [-np.eye(E), np.eye(E)], axis=1
    ).astype(ml_dtypes.bfloat16)

    with tile.TileContext(nc) as tc:
        with (
            tc.tile_pool(name="sb", bufs=1) as sb,
            tc.tile_pool(name="ps", bufs=1, space="PSUM") as ps,
        ):
            # exp/ln/copy all live in table set 6: ONE load, behind the DMA
            _load_act_set(nc, 6)

            lnbias_col = sb.tile([E, 1], F32, tag="lnbias_col")
            nc.vector.memset(lnbias_col, EPS_OUT)

            sqs = sb.tile([E, INW], F32, tag="sqs")
            nc.sync.dma_start(out=sqs, in_=in_d)
            eyes = sb.tile([E, 2 * E], BF16, tag="eyes")
            nc.gpsimd.dma_start(
                out=eyes, in_=nc.inline_tensor(eyes_np, name="c_eyes").ap()
            )
            eyeneg = eyes[:, 0:E]
            eyepos = eyes[:, E : 2 * E]

            s_col = sqs[:, 0:1]

            # bf16 copies of B (exact 0/1 values) for the +B seed matmuls
            b_bf = sb.tile([E, EP * NB], BF16, tag="b_bf")
            nc.vector.tensor_copy(b_bf, sqs[:, 1 + SQW : INW])

            # The host zeroed the sqhalf diagonal, so W diag == 1 exactly:
            # the rowsum d+1 comes straight from the Exp's fused accum_out
            # (no mask multiply, no DVE reduce), and the diag is removed in
            # the propagator via a (-I)@u matmul.  bf16 W is fine -- the
            # bf16 Gram noise dominates the error budget.
            outv = sb.tile([E, EP, NCLASSES], F32, tag="outv")
            for ep in range(EP):
                bslice = sqs[:, 1 + SQW + ep * NB : 1 + SQW + (ep + 1) * NB]
                wslice = sb.tile([E, E], BF16, tag=f"w{ep}")
                dcol = sb.tile([E, 1], F32, tag=f"d{ep}")
                if ep == 0:
                    # ep0's rowsum on DVE (idle then); skipping the fused
                    # accum saves its 187ns read on the serial ACT queue,
                    # so ep1's Exp+accum lands earlier
                    nc.scalar.activation(
                        wslice, sqs[:, 1 + ep * E : 1 + (ep + 1) * E], AF.Exp,
                        scale=s_col,
                    )
                    nc.vector.tensor_reduce(dcol, wslice, axis=AX.X, op=ALU.add)
                else:
                    nc.scalar.activation(
                        wslice, sqs[:, 1 + ep * E : 1 + (ep + 1) * E], AF.Exp,
                        scale=s_col, accum_out=dcol,
                    )
                dn = sb.tile([E, 1], F32, tag=f"dn{ep}")
                nc.vector.tensor_scalar(
                    dn, dcol, 1.0 / ALPHA, (EPS_DIAG - 1.0) / ALPHA,
                    op0=ALU.mult, op1=ALU.add,
                )
                ts = sb.tile([E, 1], F32, tag=f"t{ep}")
                nc.vector.reciprocal(ts, dn)

                u0 = sb.tile([E, NB], BF16, tag=f"u0_{ep}")
                nc.vector.tensor_scalar_mul(u0, bslice, ts)
                bb = b_bf[:, ep * NB : (ep + 1) * NB]
                u = u0
                v_ps = None
                for it in range(NEUMANN_ITERS):
                    # m = (W - I) @ u, accumulated as W@u + (-I)@u; the
                    # final step also re-adds B via I@B so v lands complete
                    # in PSUM (no DVE add on the critical path)
                    last = it == NEUMANN_ITERS - 1
                    v_ps = ps.tile([E, NB], F32, tag=f"m{ep}_{it}")
                    nc.tensor.matmul(v_ps, wslice, u, start=True, stop=False)
                    nc.tensor.matmul(v_ps, eyeneg, u, start=False, stop=not last)
                    if not last:
                        u = sb.tile([E, NB], BF16, tag=f"un{ep}_{it}")
                        nc.vector.scalar_tensor_tensor(
                            out=u, in0=v_ps, scalar=ts[:, 0:1], in1=u0,
                            op0=ALU.mult, op1=ALU.add,
                        )
                    else:
                        nc.tensor.matmul(v_ps, eyepos, bb, start=False, stop=True)
                recip_l1 = sb.tile([E, 1], F32, tag=f"rl1{ep}")
                nc.vector.reciprocal(recip_l1, v_ps[:, NCLASSES : NCLASSES + 1])
                nc.scalar.activation(
                    outv[:, ep, :], v_ps[:, 0:NCLASSES], AF.Ln,
                    bias=lnbias_col[:, 0:1], scale=recip_l1[:, 0:1],
                )
            nc.sync.dma_start(out=out_d, in_=outv)

    nc.compile()
    return nc


def _get(name, builder):
    if name not in _CACHE:
        _CACHE[name] = builder()
    return _CACHE[name]


def _prepare_l1_in_maps(x):
    x = np.asarray(x, dtype=np.float32)
    xq = x.astype(ml_dtypes.bfloat16)                       # (b, e, c)
    # xt[p, b, k, e] = xq[b, e, 128k+p]
    xt = np.ascontiguousarray(
        xq.transpose(2, 0, 1).reshape(KCHUNKS, E, B_FULL, E).transpose(1, 2, 0, 3)
    )
    # r from the quantized values so the PE diagonal cancels (to ~1e-3)
    r = (xq.astype(np.float32) ** 2).sum(axis=2, dtype=np.float64)  # (b, e)
    rneg = (-0.5 * r).astype(np.float32)
    rhi = rneg.astype(ml_dtypes.bfloat16)
    rlo = (rneg - rhi.astype(np.float32)).astype(ml_dtypes.bfloat16)
    ones = np.ones((B_FULL, E), dtype=ml_dtypes.bfloat16)
    # aug[k, b, side, e]: lhsT rows [1,1,rhi,rlo]; rhs rows [rhi,rlo,1,1]
    aug = np.ascontiguousarray(
        np.stack(
            [
                np.stack([ones, rhi], axis=1),
                np.stack([ones, rlo], axis=1),
                np.stack([rhi, ones], axis=1),
                np.stack([rlo, ones], axis=1),
            ],
            axis=0,
        )
    )  # (4, b, 2, e)
    return [
        {
            "xt": np.ascontiguousarray(xt[:, c * EP : (c + 1) * EP]),
            "aug": np.ascontiguousarray(aug[:, c * EP : (c + 1) * EP]),
        }
        for c in range(NCORES)
    ]


def _host_combine(sqh_list):
    """Global masked variance of sqhalf -> s = 1/sqrt(var).  The diag
    entries are ~1e-3 so including them in the f64 sums is harmless."""
    A = 0.0
    Qu = 0.0
    for sq in sqh_list:
        sq64 = sq.astype(np.float64)
        A += sq64.sum()
        Qu += (sq64 * sq64).sum()
    var_h = (Qu - A * A / CNT) / (CNT - 1.0)
    return np.float32(1.0 / np.sqrt(var_h))


def _prepare_l2_in_maps(res1, labels, s):
    labels = np.asarray(labels)
    bmat = np.zeros((B_FULL, E, NB), np.float32)
    bmat[..., NCLASSES] = 1.0
    for j in range(NCLASSES):
        bmat[..., j] = (labels == j).astype(np.float32)
    INW = 1 + SQW + EP * NB
    diag_idx = np.arange(E)
    maps = []
    for c in range(NCORES):
        sqs = np.empty((E, INW), np.float32)
        sqs[:, 0] = s
        sqs[:, 1 : 1 + SQW] = res1[c]["sqh"]
        # exact-zero diagonal so W_ii == 1 on device (the PE accumulation
        # order leaves ~1e-3 residue that would poison the 1e-4 damping)
        for ep in range(EP):
            sqs[diag_idx, 1 + ep * E + diag_idx] = 0.0
        sqs[:, 1 + SQW :] = (
            bmat[c * EP : (c + 1) * EP].transpose(1, 0, 2).reshape(E, EP * NB)
        )
        maps.append({"sqs": sqs})
    return maps


def _run_spmd(nc, in_maps):
    """Run with retries: a crashed predecessor process can leave the
    accelerator in NRT_EXEC_UNIT_UNRECOVERABLE; it recovers on a fresh
    attempt after a short wait."""
    import time

    last = None
    for attempt in range(3):
        try:
            return bass_utils.run_bass_kernel_spmd(
                nc, in_maps, core_ids=list(range(NCORES))
            ).results
        except Exception as e:  # noqa: BLE001 - device transients are opaque
            last = e
            time.sleep(15 * (attempt + 1))
    raise last


def run(inputs):
    nc1 = _get("l1", _build_l1)
    nc2 = _get("l2", _build_l2)
    res1 = _run_spmd(nc1, _prepare_l1_in_maps(inputs["x"]))
    s = _host_combine([r["sqh"] for r in res1])
    res2 = _run_spmd(nc2, _prepare_l2_in_maps(res1, inputs["labels"], s))
    out = np.concatenate(
        [res2[c]["out"].transpose(1, 0, 2) for c in range(NCORES)], axis=0
    )
    return np.ascontiguousarray(out.astype(np.float32))


def kernel(x, labels, nclasses):
    assert int(nclasses) == NCLASSES
    return run({"x": x, "labels": labels})


def timeline_estimate(trace_prefix=None):
    """Cost-model (TimelineSim) per-core estimates for both launches."""
    from concourse.timeline_sim import TimelineSim
    from trails.perfetto import LazyPerfetto

    for meth in ("enable_explicit_ordering", "reserve_process_order", "add_counter"):
        if not hasattr(LazyPerfetto, meth):
            setattr(LazyPerfetto, meth, lambda self, *a, **k: None)

    durs = []
    for name, builder in (("l1", _build_l1), ("l2", _build_l2)):
        nc = builder(ncores=1)
        trace = trace_prefix is not None
        tl = TimelineSim(nc, trace=trace)
        dur = tl.simulate()
        if trace and tl.perfetto is not None:
            tl.perfetto.save(f"{trace_prefix}_{name}.pftrace")
        durs.append(dur)
    return durs


if __name__ == "__main__":
    rng = np.random.default_rng(0)
    x = rng.standard_normal((B_FULL, E, C)).astype(np.float32)
    labels = rng.integers(0, NCLASSES + 1, size=(B_FULL, E))
    out = kernel(x, labels, NCLASSES)
    print("out", out.shape, out.dtype, out.min(), out.max())
